# revision 17
# baseline (speedup 1.0000x reference)
"""Trainium2 Bass kernel for nn_ChannelMerger.

Computation (per batch b):
    emb   = fourier_emb(positions[b])            # [C, 288]
    scores= emb @ heads.T                        # [C, O]
    w     = softmax(scores over C)               # [O, C]
    out[b]= w @ meg[b]                           # [O, T]

The tiny featurization/scores/softmax (B*O*C ~ 2.4M weights) is precomputed
on the host in float64; the device runs the PV merge, which is >99% of the
arithmetic and all of the HBM traffic.

Sharding: data-parallel over batch B=32 across 8 cores (4 batches/core).

Device design, from the measured PE cost law (microbench on this hw):
  - a matmul streams its OUTPUT FREE SIZE in cycles at 2.4GHz (ldweights
    fully pipelined), PROVIDED consecutive matmuls hit different PSUM banks
    (same-bank back-to-back accumulation runs at half rate) and the
    contraction tile is a full 128 partitions (K=17 tiles run at half rate).
  - out tile = [t-tile(128) partitions, O=270 free]; lhsT (stationary) is a
    [128, 128] column slice of the natural-layout meg tile, rhs (moving) is
    the transposed weight chunk [128, 270]. ~86us PE floor over the core's
    4 batches, vs ~123us for the [O-part, T-free] layout whose partial
    chunks (O=2*128+14) burn full-length streams.
  - t-tiles are processed in PAIRS with two rotating PSUM banks
    (A,B,A,B,...) so consecutive matmuls never share a bank.
  - the C remainder (273 = 2*128 + 17) is zero-padded to K=128 and runs as
    an e4m3 DoubleRow matmul (0.5 cycles/row, k-tile pair with a zero
    second tile): 135 cycles instead of 270. Weights x16 / meg /16 balances
    e4m3 underflow; measured end-to-end rel-err 1.59e-2 vs the 2e-2 gate
    (1.33e-2 with DR_REMAINDER=False, 4e-4 with MEG_FP8=False too).
  - evictions (plain f32->f16 copy; softmax 1/sum is folded into the host
    weights) alternate vector/scalar engines. Store DMAs are issued from
    the otherwise-idle Pool engine (SWDGE): a scalar.dma_start costs that
    sequencer ~667ns and starves its evictions, stalling the bank rotation.
  - output leaves the device partition-major ([b, p, g, o], t = g*128+p) so
    each store DMA writes contiguous multi-KB runs per partition; the host
    inverts the layout while casting back to f32.
  - meg travels f8e3m4 (halves the dominant read traffic; pre-scaled x2
    with the 0.5 folded into the fp16 weights).
"""

import math

import numpy as np

import concourse.bass as bass
import concourse.mybir as mybir
import concourse.tile as tile
from concourse import bacc

F32 = mybir.dt.float32
F16 = mybir.dt.float16
F8E3 = mybir.dt.float8e3
F8E4 = mybir.dt.float8e4

B, C, T = 32, 273, 8192
O, D = 270, 288
N_CORES = 8
BPC = B // N_CORES  # batches per core
MARGIN = 0.2
N_FREQ = 12
TWO_PI = 2.0 * math.pi

MEG_FP8 = True  # meg as f8e3m4 (x2 pre-scale) instead of fp16
MEG_SCALE = 2.0  # power of two; folded out via the fp16 weights
# e4m3 DoubleRow for the remainder measured ZERO gain on hw (the stream is
# still N cycles; DR fuses two k-tiles per stream, and ours was zero) while
# adding quantization error — keep False.
DR_REMAINDER = False
S3 = 1.0 / 16.0  # remainder meg pre-scale (weights carry 1/S3)

TS = 4096  # T super-tile (per-DMA free size)
NTT = TS // 128  # 128-row t-tiles per super-tile
CR = C - 256  # 17-row channel remainder, zero-padded to 128


def _build_module(meg_dt, dr) -> bass.Bass:
    r_dt = F8E4 if dr else meg_dt
    nc = bacc.Bacc()
    meg_h = nc.dram_tensor("meg", [BPC, 256, T], meg_dt, kind="ExternalInput")
    meg3_h = nc.dram_tensor("meg3", [BPC, CR, T], r_dt, kind="ExternalInput")
    # v = softmax weights, transposed, with 1/sum (and 1/MEG_SCALE) pre-folded
    v_h = nc.dram_tensor("v", [BPC, 256, O], F16, kind="ExternalInput")
    # remainder weights: [128, (2, O)] e4m3 k-tile pair (second tile zero)
    # for DoubleRow, or [128, O] zero-padded fp16 otherwise
    v3_h = nc.dram_tensor(
        "v3", [BPC, 128, (2 * O) if dr else O], F8E4 if dr else F16,
        kind="ExternalInput",
    )
    # partition-major output (t = g*128 + p); host inverts + casts f32
    out_h = nc.dram_tensor("out", [BPC, 128, T // 128, O], F16, kind="ExternalOutput")

    with tile.TileContext(nc) as tc:
        with (
            tc.tile_pool(name="const", bufs=1) as const,
            tc.tile_pool(name="megp", bufs=3) as megp,
            tc.tile_pool(name="outp", bufs=3) as outp,
            tc.tile_pool(name="psum", bufs=8, space="PSUM") as psum,
        ):
            # persistent ping-pong tiles for the 17-row meg remainder; rows
            # 17..127 zeroed once so the K=128 stream sees zero contraction
            # rows (K=17 tiles run at half rate). Zeroed FIRST, split across
            # two engines, so nothing queues behind the ~3.5us memsets.
            meg3 = []
            for s, eng in ((0, nc.vector), (1, nc.gpsimd)):
                if dr:
                    m3 = const.tile(
                        [128, 2, TS], r_dt, tag=f"meg3_{s}", name=f"meg3_{s}"
                    )
                    eng.memset(m3[:, 0, :], 0.0)
                    eng.memset(m3[:, 1, :], 0.0)
                else:
                    m3 = const.tile([128, TS], r_dt, tag=f"meg3_{s}", name=f"meg3_{s}")
                    eng.memset(m3, 0.0)
                meg3.append(m3)

            # ---- persistent weight chunks (pad rows are host zeros) ----
            # only batch 0's weights load up front; later batches' loads are
            # emitted during the previous batch so the first matmul isn't
            # queued behind 800KB of weights
            vts = [[None] * 2 for _ in range(BPC)]
            v3ts = [None] * BPC

            def load_v(b):
                for ci in range(2):
                    t_ = const.tile([128, O], F16, tag=f"v{b}_{ci}", name=f"v{b}_{ci}")
                    nc.sync.dma_start(out=t_, in_=v_h[b, ci * 128 : (ci + 1) * 128, :])
                    vts[b][ci] = t_
                if dr:
                    t3 = const.tile([128, 2, O], F8E4, tag=f"v3_{b}", name=f"v3_{b}")
                    nc.sync.dma_start(out=t3[:, 0, :], in_=v3_h[b, :, 0:O])
                    nc.sync.dma_start(out=t3[:, 1, :], in_=v3_h[b, :, O : 2 * O])
                else:
                    t3 = const.tile([128, O], F16, tag=f"v3_{b}", name=f"v3_{b}")
                    nc.sync.dma_start(out=t3, in_=v3_h[b, :, :])
                v3ts[b] = t3

            load_v(0)

            # ---- PV merge ----
            # batch 0 starts with small T-segments so the first matmul only
            # waits on ~250KB of loads instead of a full 1MB super-tile
            segs = []
            for b in range(BPC):
                tl = [1024, 1024, 2048, TS] if b == 0 else [TS, TS]
                t0 = 0
                for L in tl:
                    segs.append((b, t0, L))
                    t0 += L

            for si, (b, t0, L) in enumerate(segs):
                # prefetch the next batch's weights at the start of this
                # batch's SECOND segment: early enough to hide, late enough
                # not to delay the critical first-segment loads
                if b + 1 < BPC and si > 0 and segs[si - 1] == (b, 0, segs[si - 1][2]):
                    load_v(b + 1)
                megs = []
                for ci in range(2):
                    m_ = megp.tile([128, L], meg_dt, tag=f"meg{ci}", name=f"meg{ci}")
                    nc.sync.dma_start(
                        out=m_, in_=meg_h[b, ci * 128 : (ci + 1) * 128, t0 : t0 + L]
                    )
                    megs.append(m_)
                # remainder rows ride the SWDGE queue: their tile-reuse
                # waits must not head-of-line-block the main load queue
                m3 = meg3[si % 2]
                nc.gpsimd.dma_start(
                    out=m3[0:CR, 0, 0:L] if dr else m3[0:CR, 0:L],
                    in_=meg3_h[b, :, t0 : t0 + L],
                )

                ntt = L // 128
                g_base = t0 // 128
                # the very last segment stores every 2 pairs (4 t-tiles) so
                # the post-last-matmul drain is ~4x shorter
                sp = 2 if si == len(segs) - 1 else 4
                ostage = outp.tile([128, ntt * O], F16, tag="ostage", name="ostage")
                for pair in range(ntt // 2):
                    gA, gB = 2 * pair, 2 * pair + 1
                    psA = psum.tile([128, O], F32, tag="ps", name="psA")
                    psB = psum.tile([128, O], F32, tag="ps", name="psB")
                    # interleave the two accumulation groups so back-to-
                    # back matmuls always target different PSUM banks
                    for ci in range(2):
                        for ps, g in ((psA, gA), (psB, gB)):
                            nc.tensor.matmul(
                                ps,
                                megs[ci][:, g * 128 : (g + 1) * 128],
                                vts[b][ci],
                                start=(ci == 0),
                                stop=False,
                            )
                    for ps, g in ((psA, gA), (psB, gB)):
                        if dr:
                            nc.tensor.matmul(
                                ps,
                                m3[:, :, g * 128 : (g + 1) * 128],
                                v3ts[b],
                                start=False,
                                stop=True,
                                perf_mode=mybir.MatmulPerfMode.DoubleRow,
                            )
                        else:
                            nc.tensor.matmul(
                                ps,
                                m3[:, g * 128 : (g + 1) * 128],
                                v3ts[b],
                                start=False,
                                stop=True,
                            )
                    nc.vector.tensor_copy(ostage[:, gA * O : (gA + 1) * O], psA)
                    nc.scalar.copy(ostage[:, gB * O : (gB + 1) * O], psB)
                    if pair % sp == sp - 1:
                        # sub-supertile stores: finer store/compute overlap
                        # and a short drain tail after the last matmul
                        q0 = (pair - (sp - 1)) * 2
                        qw = sp * 2
                        nc.gpsimd.dma_start(
                            out=out_h[b, :, g_base + q0 : g_base + q0 + qw, :],
                            in_=ostage[:, q0 * O : (q0 + qw) * O],
                        )
    nc.compile()
    return nc


_MODULE_CACHE: dict = {}


def _get_module(meg_dt, dr) -> bass.Bass:
    key = (meg_dt, dr)
    if key not in _MODULE_CACHE:
        _MODULE_CACHE[key] = _build_module(meg_dt, dr)
    return _MODULE_CACHE[key]


def _host_weights(positions, heads):
    """softmax(fourier_emb(positions) @ heads.T), [B, O, C] float64."""
    freqs = (TWO_PI / (1.0 + 2.0 * MARGIN)) * np.arange(N_FREQ, dtype=np.float64)
    pos = positions.astype(np.float64) + MARGIN
    loc = (
        pos[..., 0][..., None, None] * freqs[:, None]
        + pos[..., 1][..., None, None] * freqs[None, :]
    ).reshape(B, C, N_FREQ * N_FREQ)
    emb = np.concatenate([np.cos(loc), np.sin(loc)], axis=2)  # [B, C, 2*144]
    scores = np.einsum("bcd,od->boc", emb, heads.astype(np.float64))
    scores -= scores.max(axis=2, keepdims=True)
    e = np.exp(scores)
    return e / e.sum(axis=2, keepdims=True)  # [B, O, C]


def _host_prep(meg, positions, heads):
    """Shard + lay out inputs for the 8 cores."""
    w = _host_weights(positions, heads)  # [B, O, C]
    vmain = w.transpose(0, 2, 1)[:, :256, :]  # [B, 256, O]
    w3 = w.transpose(0, 2, 1)[:, 256:, :]  # [B, CR, O]

    if MEG_FP8:
        import ml_dtypes

        vmain = vmain / MEG_SCALE
        meg_dev = (meg[:, :256, :] * np.float32(MEG_SCALE)).astype(
            ml_dtypes.float8_e3m4
        )
    else:
        meg_dev = meg[:, :256, :].astype(np.float16)
    v16 = vmain.astype(np.float16)

    if DR_REMAINDER:
        import ml_dtypes

        meg3_dev = (meg[:, 256:, :] * np.float32(S3)).astype(ml_dtypes.float8_e4m3)
        v3 = np.zeros((B, 128, 2 * O), np.float32)
        v3[:, :CR, :O] = w3 / S3
        v3_dev = v3.astype(ml_dtypes.float8_e4m3)
    else:
        meg3_dev = (
            (meg[:, 256:, :] * np.float32(MEG_SCALE)).astype(ml_dtypes.float8_e3m4)
            if MEG_FP8
            else meg[:, 256:, :].astype(np.float16)
        )
        v3 = np.zeros((B, 128, O), np.float16)
        v3[:, :CR, :] = (w3 / (MEG_SCALE if MEG_FP8 else 1.0)).astype(np.float16)
        v3_dev = v3

    in_maps = []
    for k in range(N_CORES):
        sl = slice(k * BPC, (k + 1) * BPC)
        in_maps.append(
            {
                "meg": np.ascontiguousarray(meg_dev[sl]),
                "meg3": np.ascontiguousarray(meg3_dev[sl]),
                "v": np.ascontiguousarray(v16[sl]),
                "v3": np.ascontiguousarray(v3_dev[sl]),
            }
        )
    return in_maps


LAST_RESULTS = None  # BassKernelResults of the most recent kernel() call


def kernel(meg: np.ndarray, positions: np.ndarray, heads: np.ndarray) -> np.ndarray:
    global LAST_RESULTS
    from concourse.bass_utils import run_bass_kernel_spmd

    nc = _get_module(F8E3 if MEG_FP8 else F16, DR_REMAINDER)
    in_maps = _host_prep(
        np.asarray(meg, dtype=np.float32),
        np.asarray(positions, dtype=np.float32),
        np.asarray(heads, dtype=np.float32),
    )
    res = run_bass_kernel_spmd(nc, in_maps, core_ids=list(range(N_CORES)))
    LAST_RESULTS = res
    # [BPC, 128, 64, O] f16 (t = g*128 + p) -> [BPC, O, T] f32
    out = np.empty((B, O, T), np.float32)
    for k, r in enumerate(res.results):
        x = r["out"]  # [BPC, 128, 64, O]
        for b in range(BPC):
            out[k * BPC + b] = x[b].transpose(2, 1, 0).reshape(O, T)
    return out


# revision 18
# speedup vs baseline: 1.0709x; 1.0709x over previous
"""Trainium2 Bass kernel for nn_ChannelMerger.

Computation (per batch b):
    emb   = fourier_emb(positions[b])            # [C, 288]
    scores= emb @ heads.T                        # [C, O]
    w     = softmax(scores over C)               # [O, C]
    out[b]= w @ meg[b]                           # [O, T]

The tiny featurization/scores/softmax (B*O*C ~ 2.4M weights) is precomputed
on the host in float64; the device runs the PV merge, which is >99% of the
arithmetic and all of the HBM traffic.

Sharding: data-parallel over batch B=32 across 8 cores (4 batches/core).

Device design, from the measured PE cost law (microbench on this hw):
  - a matmul streams its OUTPUT FREE SIZE in cycles at 2.4GHz (ldweights
    fully pipelined), PROVIDED consecutive matmuls hit different PSUM banks
    (same-bank back-to-back accumulation runs at half rate) and the
    contraction tile is a full 128 partitions (K=17 tiles run at half rate).
  - out tile = [t-tile(128) partitions, O=270 free]; lhsT (stationary) is a
    [128, 128] column slice of the natural-layout meg tile, rhs (moving) is
    the transposed weight chunk [128, 270]. ~86us PE floor over the core's
    4 batches, vs ~123us for the [O-part, T-free] layout whose partial
    chunks (O=2*128+14) burn full-length streams.
  - t-tiles are processed in PAIRS with two rotating PSUM banks
    (A,B,A,B,...) so consecutive matmuls never share a bank.
  - the C remainder (273 = 2*128 + 17) is zero-padded to K=128 and runs as
    an e4m3 DoubleRow matmul (0.5 cycles/row, k-tile pair with a zero
    second tile): 135 cycles instead of 270. Weights x16 / meg /16 balances
    e4m3 underflow; measured end-to-end rel-err 1.59e-2 vs the 2e-2 gate
    (1.33e-2 with DR_REMAINDER=False, 4e-4 with MEG_FP8=False too).
  - evictions (plain f32->f16 copy; softmax 1/sum is folded into the host
    weights) alternate vector/scalar engines. Store DMAs are issued from
    the otherwise-idle Pool engine (SWDGE): a scalar.dma_start costs that
    sequencer ~667ns and starves its evictions, stalling the bank rotation.
  - output leaves the device partition-major ([b, p, g, o], t = g*128+p) so
    each store DMA writes contiguous multi-KB runs per partition; the host
    inverts the layout while casting back to f32.
  - meg travels f8e3m4 (halves the dominant read traffic; pre-scaled x2
    with the 0.5 folded into the fp16 weights).
"""

import math

import numpy as np

import concourse.bass as bass
import concourse.mybir as mybir
import concourse.tile as tile
from concourse import bacc

F32 = mybir.dt.float32
F16 = mybir.dt.float16
F8E3 = mybir.dt.float8e3
F8E4 = mybir.dt.float8e4

B, C, T = 32, 273, 8192
O, D = 270, 288
N_CORES = 8
BPC = B // N_CORES  # batches per core
MARGIN = 0.2
N_FREQ = 12
TWO_PI = 2.0 * math.pi

MEG_FP8 = True  # meg as f8e3m4 (x2 pre-scale) instead of fp16
MEG_SCALE = 2.0  # power of two; folded out via the fp16 weights
# e4m3 DoubleRow for the remainder measured ZERO gain on hw (the stream is
# still N cycles; DR fuses two k-tiles per stream, and ours was zero) while
# adding quantization error — keep False.
DR_REMAINDER = False
S3 = 1.0 / 16.0  # remainder meg pre-scale (weights carry 1/S3)

TS = 4096  # T super-tile (per-DMA free size)
NTT = TS // 128  # 128-row t-tiles per super-tile
CR = C - 256  # 17-row channel remainder, zero-padded to 128


def _build_module(meg_dt, dr) -> bass.Bass:
    r_dt = F8E4 if dr else meg_dt
    nc = bacc.Bacc()
    meg_h = nc.dram_tensor("meg", [BPC, 256, T], meg_dt, kind="ExternalInput")
    meg3_h = nc.dram_tensor("meg3", [BPC, CR, T], r_dt, kind="ExternalInput")
    # v = softmax weights, transposed, with 1/sum (and 1/MEG_SCALE) pre-folded
    v_h = nc.dram_tensor("v", [BPC, 256, O], F16, kind="ExternalInput")
    # remainder weights: [128, (2, O)] e4m3 k-tile pair (second tile zero)
    # for DoubleRow, or [128, O] zero-padded fp16 otherwise
    v3_h = nc.dram_tensor(
        "v3", [BPC, 128, (2 * O) if dr else O], F8E4 if dr else F16,
        kind="ExternalInput",
    )
    # partition-major output (t = g*128 + p); host inverts + casts f32
    out_h = nc.dram_tensor("out", [BPC, 128, T // 128, O], F16, kind="ExternalOutput")

    with tile.TileContext(nc) as tc:
        with (
            tc.tile_pool(name="const", bufs=1) as const,
            tc.tile_pool(name="megp", bufs=4) as megp,
            tc.tile_pool(name="outp", bufs=3) as outp,
            tc.tile_pool(name="psum", bufs=8, space="PSUM") as psum,
        ):
            # persistent ping-pong tiles for the 17-row meg remainder; rows
            # 17..127 zeroed once so the K=128 stream sees zero contraction
            # rows (K=17 tiles run at half rate). Zeroed FIRST, split across
            # two engines, so nothing queues behind the ~3.5us memsets.
            meg3 = []
            for s, eng in ((0, nc.vector), (1, nc.gpsimd)):
                if dr:
                    m3 = const.tile(
                        [128, 2, TS], r_dt, tag=f"meg3_{s}", name=f"meg3_{s}"
                    )
                    eng.memset(m3[:, 0, :], 0.0)
                    eng.memset(m3[:, 1, :], 0.0)
                else:
                    m3 = const.tile([128, TS], r_dt, tag=f"meg3_{s}", name=f"meg3_{s}")
                    eng.memset(m3, 0.0)
                meg3.append(m3)

            # ---- persistent weight chunks (pad rows are host zeros) ----
            # only batch 0's weights load up front; later batches' loads are
            # emitted during the previous batch so the first matmul isn't
            # queued behind 800KB of weights
            vts = [[None] * 2 for _ in range(BPC)]
            v3ts = [None] * BPC

            def load_v(b):
                for ci in range(2):
                    t_ = const.tile([128, O], F16, tag=f"v{b}_{ci}", name=f"v{b}_{ci}")
                    nc.sync.dma_start(out=t_, in_=v_h[b, ci * 128 : (ci + 1) * 128, :])
                    vts[b][ci] = t_
                if dr:
                    t3 = const.tile([128, 2, O], F8E4, tag=f"v3_{b}", name=f"v3_{b}")
                    nc.sync.dma_start(out=t3[:, 0, :], in_=v3_h[b, :, 0:O])
                    nc.sync.dma_start(out=t3[:, 1, :], in_=v3_h[b, :, O : 2 * O])
                else:
                    t3 = const.tile([128, O], F16, tag=f"v3_{b}", name=f"v3_{b}")
                    nc.sync.dma_start(out=t3, in_=v3_h[b, :, :])
                v3ts[b] = t3

            load_v(0)

            # ---- PV merge ----
            # batch 0 starts with small T-segments so the first matmul only
            # waits on ~250KB of loads instead of a full 1MB super-tile
            segs = []
            for b in range(BPC):
                tl = [1024, 1024, 2048, TS] if b == 0 else [TS, TS]
                t0 = 0
                for L in tl:
                    segs.append((b, t0, L))
                    t0 += L

            for si, (b, t0, L) in enumerate(segs):
                # prefetch the next batch's weights at the start of this
                # batch's SECOND segment: early enough to hide, late enough
                # not to delay the critical first-segment loads
                if b + 1 < BPC and si > 0 and segs[si - 1] == (b, 0, segs[si - 1][2]):
                    load_v(b + 1)
                megs = []
                for ci in range(2):
                    m_ = megp.tile([128, L], meg_dt, tag=f"meg{ci}", name=f"meg{ci}")
                    nc.sync.dma_start(
                        out=m_, in_=meg_h[b, ci * 128 : (ci + 1) * 128, t0 : t0 + L]
                    )
                    megs.append(m_)
                # remainder rows ride the SWDGE queue: their tile-reuse
                # waits must not head-of-line-block the main load queue
                m3 = meg3[si % 2]
                nc.gpsimd.dma_start(
                    out=m3[0:CR, 0, 0:L] if dr else m3[0:CR, 0:L],
                    in_=meg3_h[b, :, t0 : t0 + L],
                )

                ntt = L // 128
                g_base = t0 // 128
                # the very last segment stores every 2 pairs (4 t-tiles) so
                # the post-last-matmul drain is ~4x shorter
                sp = 2 if si == len(segs) - 1 else 4
                ostage = outp.tile([128, ntt * O], F16, tag="ostage", name="ostage")
                for pair in range(ntt // 2):
                    gA, gB = 2 * pair, 2 * pair + 1
                    psA = psum.tile([128, O], F32, tag="ps", name="psA")
                    psB = psum.tile([128, O], F32, tag="ps", name="psB")
                    # interleave the two accumulation groups so back-to-
                    # back matmuls always target different PSUM banks
                    for ci in range(2):
                        for ps, g in ((psA, gA), (psB, gB)):
                            nc.tensor.matmul(
                                ps,
                                megs[ci][:, g * 128 : (g + 1) * 128],
                                vts[b][ci],
                                start=(ci == 0),
                                stop=False,
                            )
                    for ps, g in ((psA, gA), (psB, gB)):
                        if dr:
                            nc.tensor.matmul(
                                ps,
                                m3[:, :, g * 128 : (g + 1) * 128],
                                v3ts[b],
                                start=False,
                                stop=True,
                                perf_mode=mybir.MatmulPerfMode.DoubleRow,
                            )
                        else:
                            nc.tensor.matmul(
                                ps,
                                m3[:, g * 128 : (g + 1) * 128],
                                v3ts[b],
                                start=False,
                                stop=True,
                            )
                    nc.vector.tensor_copy(ostage[:, gA * O : (gA + 1) * O], psA)
                    nc.scalar.copy(ostage[:, gB * O : (gB + 1) * O], psB)
                    if pair % sp == sp - 1:
                        # sub-supertile stores: finer store/compute overlap
                        # and a short drain tail after the last matmul
                        q0 = (pair - (sp - 1)) * 2
                        qw = sp * 2
                        nc.gpsimd.dma_start(
                            out=out_h[b, :, g_base + q0 : g_base + q0 + qw, :],
                            in_=ostage[:, q0 * O : (q0 + qw) * O],
                        )
    nc.compile()
    return nc


_MODULE_CACHE: dict = {}


def _get_module(meg_dt, dr) -> bass.Bass:
    key = (meg_dt, dr)
    if key not in _MODULE_CACHE:
        _MODULE_CACHE[key] = _build_module(meg_dt, dr)
    return _MODULE_CACHE[key]


def _host_weights(positions, heads):
    """softmax(fourier_emb(positions) @ heads.T), [B, O, C] float64."""
    freqs = (TWO_PI / (1.0 + 2.0 * MARGIN)) * np.arange(N_FREQ, dtype=np.float64)
    pos = positions.astype(np.float64) + MARGIN
    loc = (
        pos[..., 0][..., None, None] * freqs[:, None]
        + pos[..., 1][..., None, None] * freqs[None, :]
    ).reshape(B, C, N_FREQ * N_FREQ)
    emb = np.concatenate([np.cos(loc), np.sin(loc)], axis=2)  # [B, C, 2*144]
    scores = np.einsum("bcd,od->boc", emb, heads.astype(np.float64))
    scores -= scores.max(axis=2, keepdims=True)
    e = np.exp(scores)
    return e / e.sum(axis=2, keepdims=True)  # [B, O, C]


def _host_prep(meg, positions, heads):
    """Shard + lay out inputs for the 8 cores."""
    w = _host_weights(positions, heads)  # [B, O, C]
    vmain = w.transpose(0, 2, 1)[:, :256, :]  # [B, 256, O]
    w3 = w.transpose(0, 2, 1)[:, 256:, :]  # [B, CR, O]

    if MEG_FP8:
        import ml_dtypes

        vmain = vmain / MEG_SCALE
        meg_dev = (meg[:, :256, :] * np.float32(MEG_SCALE)).astype(
            ml_dtypes.float8_e3m4
        )
    else:
        meg_dev = meg[:, :256, :].astype(np.float16)
    v16 = vmain.astype(np.float16)

    if DR_REMAINDER:
        import ml_dtypes

        meg3_dev = (meg[:, 256:, :] * np.float32(S3)).astype(ml_dtypes.float8_e4m3)
        v3 = np.zeros((B, 128, 2 * O), np.float32)
        v3[:, :CR, :O] = w3 / S3
        v3_dev = v3.astype(ml_dtypes.float8_e4m3)
    else:
        meg3_dev = (
            (meg[:, 256:, :] * np.float32(MEG_SCALE)).astype(ml_dtypes.float8_e3m4)
            if MEG_FP8
            else meg[:, 256:, :].astype(np.float16)
        )
        v3 = np.zeros((B, 128, O), np.float16)
        v3[:, :CR, :] = (w3 / (MEG_SCALE if MEG_FP8 else 1.0)).astype(np.float16)
        v3_dev = v3

    in_maps = []
    for k in range(N_CORES):
        sl = slice(k * BPC, (k + 1) * BPC)
        in_maps.append(
            {
                "meg": np.ascontiguousarray(meg_dev[sl]),
                "meg3": np.ascontiguousarray(meg3_dev[sl]),
                "v": np.ascontiguousarray(v16[sl]),
                "v3": np.ascontiguousarray(v3_dev[sl]),
            }
        )
    return in_maps


LAST_RESULTS = None  # BassKernelResults of the most recent kernel() call


def kernel(meg: np.ndarray, positions: np.ndarray, heads: np.ndarray) -> np.ndarray:
    global LAST_RESULTS
    from concourse.bass_utils import run_bass_kernel_spmd

    nc = _get_module(F8E3 if MEG_FP8 else F16, DR_REMAINDER)
    in_maps = _host_prep(
        np.asarray(meg, dtype=np.float32),
        np.asarray(positions, dtype=np.float32),
        np.asarray(heads, dtype=np.float32),
    )
    res = run_bass_kernel_spmd(nc, in_maps, core_ids=list(range(N_CORES)))
    LAST_RESULTS = res
    # [BPC, 128, 64, O] f16 (t = g*128 + p) -> [BPC, O, T] f32
    out = np.empty((B, O, T), np.float32)
    for k, r in enumerate(res.results):
        x = r["out"]  # [BPC, 128, 64, O]
        for b in range(BPC):
            out[k * BPC + b] = x[b].transpose(2, 1, 0).reshape(O, T)
    return out
